# revision 1
# baseline (speedup 1.0000x reference)
"""Bidirectional GATConv + fusion + BatchNorm + ReLU on 8 Trainium2 cores.

Strategy: nodes sharded 8 ways by aggregation target. Each core:
  1. projects x -> h_f/h_b + attention logits (replicated compute, bf16 PE),
     writing gather tables [h | a_s] (768B rows) and a local a_d table.
  2. walks its incident edges (dst-sorted, host-partitioned) in 128-edge
     chunks: dma_gather of source rows, softmax weights via exp(lrelu),
     scatter-add into PSUM via one-hot matmul (lhsT = S, built on DVE by
     iota==dstpos compare).
  3. fuses [fwd|bwd] @ W_fuse, computes BN stats, AllReduces them (4KB),
     normalizes + ReLU, writes its 1/8 output shard.
Biases provably cancel through BatchNorm and are dropped.
"""
import sys

sys.path.insert(0, "/opt/trn_rl_repo")

import numpy as np
import ml_dtypes

import concourse.bass as bass
import concourse.bacc as bacc
import concourse.mybir as mybir
from concourse import tile
from concourse import library_config
from concourse.bass_utils import run_bass_kernel_spmd

bf16 = mybir.dt.bfloat16
f32 = mybir.dt.float32
i16 = mybir.dt.int16
Alu = mybir.AluOpType
Act = mybir.ActivationFunctionType

NCORES = 8
USE_CC = __import__("os").environ.get("NO_CC", "0") != "1"
NO_FUSE = __import__("os").environ.get("NO_FUSE", "0") == "1"
NO_EDGE = __import__("os").environ.get("NO_EDGE", "0") == "1"
KB = 4          # dst blocks per gather supergroup
NEG_SLOPE = 0.2
BN_EPS = 1e-5
DUMMY_AS = -60.0


def _derive(n_nodes):
    npc = n_nodes // NCORES
    nb = (npc + 127) // 128
    half = ((n_nodes // 2) // 128) * 128
    trows_lo = half + 64           # dummy row at index `half`
    trows_hi = (n_nodes - half) + NCORES * 16 + 64  # covers proj padding rows
    return npc, nb, half, trows_lo, trows_hi


def _pack_idx(arr):
    """int16 [n] (n%16==0) -> [128, n/16] wrapped in 16 partitions, replicated per Q7 core."""
    a = arr.reshape(-1, 16).T
    return np.tile(a, (8, 1)).astype(np.int16)


def _prep_edges(gidx, anode, n_nodes):
    """Host edge partitioning for one direction.

    gidx: gather-side node per edge; anode: aggregation node per edge.
    Returns per-core chunk tensors with a uniform (CLO, CHI) template.
    """
    npc, nb, half, _, _ = _derive(n_nodes)
    n_all = ((n_nodes + 127) // 128) * 128
    dlo, dhi = half + 16, (n_all - half) + 16
    core = anode // npc
    local = anode - core * npc
    block = local // 128
    dstpos = local % 128
    hi = (gidx >= half).astype(np.int64)
    lidx = gidx - hi * half

    counts = np.zeros((NCORES, nb, 2), dtype=np.int64)
    np.add.at(counts, (core, block, hi), 1)
    nchunks = -(-counts // 128)  # ceil
    clo = int(nchunks[:, :, 0].max())
    chi = int(nchunks[:, :, 1].max())

    order = np.lexsort((lidx, hi, block, core))
    g_s, blk_s, hi_s, lidx_s, dp_s = (
        core[order], block[order], hi[order], lidx[order], dstpos[order])

    cmax = max(clo, chi)
    g1 = np.empty((NCORES, nb, 2, cmax * 128), dtype=np.int16)
    g1[:, :, 0, :] = dlo
    g1[:, :, 1, :] = dhi
    g2 = np.zeros((NCORES, nb, 2, cmax * 128), dtype=np.int16)
    dp = np.full((NCORES, nb, 2, cmax * 128), 200.0, dtype=ml_dtypes.bfloat16)

    # fill positions within each (core, block, hi) group
    flat_grp = (g_s * nb + blk_s) * 2 + hi_s
    # positions via cumcount
    idx_sorted = np.argsort(flat_grp, kind="stable")
    fg = flat_grp[idx_sorted]
    pos = np.arange(len(fg)) - np.concatenate(([0], np.cumsum(np.bincount(fg, minlength=NCORES*nb*2))))[fg]
    c_, b_, h_ = fg // (nb * 2), (fg // 2) % nb, fg % 2
    g1[c_, b_, h_, pos] = lidx_s[idx_sorted].astype(np.int16)
    g2[c_, b_, h_, pos] = (blk_s[idx_sorted] * 128 + dp_s[idx_sorted]).astype(np.int16)
    g2[:, :, :, :][g2 < 0] = 0
    # pad g2 entries point at the block's first row
    padmask = np.ones((NCORES, nb, 2, cmax * 128), dtype=bool)
    padmask[c_, b_, h_, pos] = False
    bb = np.broadcast_to(np.arange(nb)[None, :, None, None] * 128,
                         (NCORES, nb, 2, cmax * 128))
    g2[padmask] = bb[padmask].astype(np.int16)
    dp[c_, b_, h_, pos] = dp_s[idx_sorted].astype(ml_dtypes.bfloat16)

    # trim halves to their own chunk counts
    g1lo, g1hi = g1[:, :, 0, :clo * 128], g1[:, :, 1, :chi * 128]
    g2lo, g2hi = g2[:, :, 0, :clo * 128], g2[:, :, 1, :chi * 128]
    dplo, dphi = dp[:, :, 0, :clo * 128], dp[:, :, 1, :chi * 128]

    # gather idx streams: per supergroup: lo idxs then hi idxs (block-major)
    g1_streams, g2_streams = [], []
    for c in range(NCORES):
        p1, p2 = [], []
        for bs in range(0, nb, KB):
            be = min(bs + KB, nb)
            p1 += [_pack_idx(g1lo[c, bs:be].ravel()), _pack_idx(g1hi[c, bs:be].ravel())]
            p2 += [_pack_idx(g2lo[c, bs:be].ravel()), _pack_idx(g2hi[c, bs:be].ravel())]
        g1_streams.append(np.concatenate(p1, axis=1))
        g2_streams.append(np.concatenate(p2, axis=1))
    # dstpos stream [128, nb*(clo+chi)] block-major, lo chunks then hi chunks
    dpl = dplo.reshape(NCORES, nb, clo, 128).transpose(0, 3, 1, 2).reshape(NCORES, 128, nb * clo)
    dph = dphi.reshape(NCORES, nb, chi, 128).transpose(0, 3, 1, 2).reshape(NCORES, 128, nb * chi)
    dp_stream = np.concatenate([dpl, dph], axis=2)
    return clo, chi, np.stack(g1_streams), np.stack(g2_streams), np.ascontiguousarray(dp_stream)


def _build_program(n_nodes, clo_f, chi_f, clo_b, chi_b):
    npc, nb, half, trows_lo, trows_hi = _derive(n_nodes)
    nbr = npc - (nb - 1) * 128            # rows in last block
    npad = nb * 128
    n_all = ((n_nodes + 127) // 128) * 128
    npb = n_all // 128                     # projection node blocks
    dummy_lo = half + 16
    dummy_hi = (n_all - half) + 16

    nc = bacc.Bacc(None, target_bir_lowering=False)
    inp = lambda name, shape, dt: nc.declare_dram_parameter(name, shape, dt, isOutput=False)
    xT = inp("xT", [512, n_all], bf16)
    xTo = inp("xTo", [512, npad], bf16)
    wall = inp("wall", [512, 528], bf16)
    wfuse = inp("wfuse", [512, 512], bf16)
    drow = inp("drow", [1, 384], bf16)
    iota_in = inp("iota", [128, 128], bf16)
    ident = inp("ident", [128, 128], f32)
    bnpg = inp("bnpg", [32, 128], f32)
    bnpb = inp("bnpb", [32, 128], f32)
    streams = {}
    for d, (clo, chi) in (("f", (clo_f, chi_f)), ("b", (clo_b, chi_b))):
        tc_d = nb * (clo + chi)
        streams["g1" + d] = inp("g1" + d, [128, tc_d * 8], i16)
        streams["g2" + d] = inp("g2" + d, [128, tc_d * 8], i16)
        streams["dp" + d] = inp("dp" + d, [128, tc_d], f32)
    out_d = nc.declare_dram_parameter("out", [npc, 512], f32, isOutput=True)

    tabs = {d: [nc.dram_tensor(f"tab{d}{h}", [tr, 384], bf16)
                for h, tr in (("lo", trows_lo), ("hi", trows_hi))] for d in "fb"}
    adtab = nc.dram_tensor("adtab", [npad, 128], bf16)
    combined = nc.dram_tensor("combined", [npad, 512], bf16)
    ccin = nc.dram_tensor("ccin", [128, 64], f32)
    ccout = nc.dram_tensor("ccout", [128, 64], f32, addr_space="Shared")
    abtmp = nc.dram_tensor("abtmp", [8, 128], f32)

    hblocks = half // 128

    with tile.TileContext(nc) as tc:
        with (
            tc.tile_pool(name="const", bufs=1) as cpool,
        ):
            nc.gpsimd.load_library(library_config.mlp)
            wall_sb = cpool.tile([128, 4, 528], bf16)
            for k in range(4):
                nc.sync.dma_start(wall_sb[:, k, :], wall[k * 128:(k + 1) * 128, :])
            iota_sb = cpool.tile([128, 128], bf16)
            nc.sync.dma_start(iota_sb[:], iota_in[:])
            # zero-fill unwritten table tail rows, then dummy rows
            zt = cpool.tile([128, 384], bf16)
            nc.gpsimd.memset(zt[:], 0.0)
            for b in range(npad // 128):
                nc.sync.dma_start(adtab[b * 128:(b + 1) * 128, :], zt[:, 0:128])
            for d in "fb":
                r = half
                while r < trows_lo:
                    n = min(128, trows_lo - r)
                    nc.sync.dma_start(tabs[d][0][r:r + n, :], zt[0:n, :])
                    r += n
                r = n_all - half
                while r < trows_hi:
                    n = min(128, trows_hi - r)
                    nc.sync.dma_start(tabs[d][1][r:r + n, :], zt[0:n, :])
                    r += n
                nc.sync.dma_start(tabs[d][0][dummy_lo:dummy_lo + 1, :], drow[:])
                nc.sync.dma_start(tabs[d][1][dummy_hi:dummy_hi + 1, :], drow[:])

            # ---------------- projection ----------------
            with (tc.tile_pool(name="proj", bufs=3) as pj,
                  tc.tile_pool(name="pspj", bufs=2, space="PSUM") as pp):
                for nbk in range(npb):
                    xt = pj.tile([128, 4, 128], bf16, tag="xt")
                    nc.sync.dma_start(
                        xt[:], xT[:, nbk * 128:(nbk + 1) * 128]
                        .rearrange("(k p) n -> p k n", p=128))
                    ps = {d: pp.tile([128, 260], f32, tag="pj" + d, name=f"ps{d}_{nbk}")
                          for d in "fb"}
                    for k in range(4):
                        nc.tensor.matmul(ps["f"][:], xt[:, k, :], wall_sb[:, k, 0:260],
                                         start=(k == 0), stop=(k == 3))
                    for k in range(4):
                        nc.tensor.matmul(ps["b"][:], xt[:, k, :], wall_sb[:, k, 260:520],
                                         start=(k == 0), stop=(k == 3))
                    for d, eng in (("f", nc.scalar), ("b", nc.vector)):
                        st = pj.tile([128, 384], bf16, tag="st" + d)
                        if d == "f":
                            eng.activation(st[:, 0:260], ps[d][:], Act.Copy)
                        else:
                            eng.tensor_copy(st[:, 0:260], ps[d][:])
                        nc.gpsimd.memset(st[:, 260:384], 0.0)
                        if nbk < hblocks:
                            dst = tabs[d][0][nbk * 128:(nbk + 1) * 128, :]
                        else:
                            r0 = (nbk - hblocks) * 128
                            dst = tabs[d][1][r0:r0 + 128, :]
                        nc.sync.dma_start(dst, st[:])
                # local a_d table
                ad_stage = pj.tile([128, nb, 8], bf16, tag="ad")
                for b in range(nb):
                    xo = pj.tile([128, 4, 128], bf16, tag="xo")
                    nc.sync.dma_start(
                        xo[:], xTo[:, b * 128:(b + 1) * 128]
                        .rearrange("(k p) n -> p k n", p=128))
                    pa = pp.tile([128, 8], f32, tag="pa", bufs=1)
                    for k in range(4):
                        nc.tensor.matmul(pa[:], xo[:, k, :], wall_sb[:, k, 520:528],
                                         start=(k == 0), stop=(k == 3))
                    nc.vector.tensor_copy(ad_stage[:, b, :], pa[:])
                nc.sync.dma_start(
                    adtab.rearrange("(b p) c -> p b c", p=128)[:, :, 0:8], ad_stage[:])

            # ---------------- edge passes ----------------
            edirs = () if NO_EDGE else (("f", clo_f, chi_f, 0), ("b", clo_b, chi_b, 256))
            for d, clo, chi, dcol in edirs:
                tc_d = nb * (clo + chi)
                with tc.tile_pool(name="edge" + d, bufs=1) as ep:
                    dp_sb = ep.tile([128, tc_d], f32)
                    nc.sync.dma_start(dp_sb[:], streams["dp" + d][:])
                    g1_sb = ep.tile([128, tc_d * 8], i16)
                    nc.sync.dma_start(g1_sb[:], streams["g1" + d][:])
                    g2_sb = ep.tile([128, tc_d * 8], i16)
                    nc.sync.dma_start(g2_sb[:], streams["g2" + d][:])
                    with (tc.tile_pool(name="ew" + d, bufs=2) as ew,
                      tc.tile_pool(name="psed" + d, bufs=4, space="PSUM") as pp):
                        c1 = c2 = 0  # stream column cursors
                        for bs in range(0, nb, KB):
                            kbs = min(KB, nb - bs)
                            tiles = {}
                            for hname, cc, tabx in (("lo", clo, tabs[d][0]),
                                                    ("hi", chi, tabs[d][1])):
                                ni = kbs * cc * 128
                                gt = ew.tile([128, kbs * cc, 384], bf16, tag="g1t" + hname)
                                nc.gpsimd.dma_gather(
                                    gt[:], tabx[:], g1_sb[:, c1:c1 + ni // 16],
                                    num_idxs=ni, num_idxs_reg=ni, elem_size=384, single_packet=False)
                                c1 += ni // 16
                                at = ew.tile([128, kbs * cc, 128], bf16, tag="g2t" + hname)
                                nc.gpsimd.dma_gather(
                                    at[:], adtab[:], g2_sb[:, c2:c2 + ni // 16],
                                    num_idxs=ni, num_idxs_reg=ni, elem_size=128, single_packet=False)
                                c2 += ni // 16
                                tiles[hname] = (gt, at, cc)
                            for j in range(kbs):
                                b = bs + j
                                pb = pp.tile([128, 260], f32, tag="pb")
                                first = True
                                for hname in ("lo", "hi"):
                                    gt, at, cc = tiles[hname]
                                    if cc == 0:
                                        continue
                                    hofs = 0 if hname == "lo" else nb * clo
                                    tc0 = hofs + b * cc
                                    adofs = dcol // 256 * 4  # f: cols 0:4, b: 4:8
                                    et = ew.tile([128, cc, 4], bf16, tag="et")
                                    for k in range(cc):
                                        nc.vector.tensor_tensor(
                                            et[:, k, :], gt[:, j * cc + k, 256:260],
                                            at[:, j * cc + k, adofs:adofs + 4], Alu.add)
                                    lt2 = ew.tile([128, cc, 4], bf16, tag="lt2")
                                    nc.vector.tensor_scalar_mul(
                                        lt2[:].rearrange("p c h -> p (c h)"),
                                        et[:].rearrange("p c h -> p (c h)"), NEG_SLOPE)
                                    lt = ew.tile([128, cc, 4], f32, tag="lt")
                                    nc.vector.tensor_tensor(
                                        lt[:].rearrange("p c h -> p (c h)"),
                                        et[:].rearrange("p c h -> p (c h)"),
                                        lt2[:].rearrange("p c h -> p (c h)"), Alu.max)
                                    ext = ew.tile([128, cc, 4], f32, tag="ext")
                                    nc.scalar.activation(ext[:], lt[:], Act.Exp)
                                    mt = ew.tile([128, cc, 260], bf16, tag="mt")
                                    for k in range(cc):
                                        for h in range(2):
                                            nc.vector.tensor_scalar(
                                                mt[:, k, h * 64:(h + 1) * 64],
                                                gt[:, j * cc + k, h * 64:(h + 1) * 64],
                                                ext[:, k, h:h + 1], None, op0=Alu.mult)
                                        for h in range(2, 4):
                                            nc.scalar.activation(
                                                mt[:, k, h * 64:(h + 1) * 64],
                                                gt[:, j * cc + k, h * 64:(h + 1) * 64],
                                                Act.Copy, scale=ext[:, k, h:h + 1])
                                    for k in range(cc):
                                        nc.scalar.activation(mt[:, k, 256:260],
                                                             ext[:, k, :], Act.Copy)
                                    st = ew.tile([128, cc, 128], bf16, tag="st")
                                    for k in range(cc):
                                        nc.vector.tensor_scalar(
                                            st[:, k, :], iota_sb[:],
                                            dp_sb[:, tc0 + k:tc0 + k + 1], None,
                                            op0=Alu.is_equal)
                                    for k in range(cc):
                                        last = (hname == "hi" or chi == 0) and k == cc - 1
                                        nc.tensor.matmul(pb[:], st[:, k, :], mt[:, k, :],
                                                         start=first, stop=last)
                                        first = False
                                dn = ew.tile([128, 4], f32, tag="dn")
                                nc.vector.tensor_scalar_add(dn[:], pb[:, 256:260], 1e-16)
                                rc = ew.tile([128, 4], f32, tag="rc")
                                nc.vector.reciprocal(rc[:], dn[:])
                                ob = ew.tile([128, 256], bf16, tag="ob")
                                for h in range(4):
                                    nc.vector.tensor_scalar(
                                        ob[:, h * 64:(h + 1) * 64], pb[:, h * 64:(h + 1) * 64],
                                        rc[:, h:h + 1], None, op0=Alu.mult)
                                nc.sync.dma_start(
                                    combined[b * 128:(b + 1) * 128, dcol:dcol + 256], ob[:])

            # ---------------- fusion + BN ----------------
            if NO_FUSE:
                with tc.tile_pool(name="nf", bufs=1) as nf:
                    z = nf.tile([128, 512], f32)
                    nc.vector.memset(z[:], 0.0)
                    for b in range(nb):
                        rows = min(128, npc - b * 128)
                        nc.sync.dma_start(out_d[b * 128:b * 128 + rows, :], z[0:rows, :])
            if not NO_FUSE:
             with (tc.tile_pool(name="fuse", bufs=1) as fp,
                  tc.tile_pool(name="psfu", bufs=1, space="PSUM") as pp):
                wf_sb = fp.tile([128, 4, 512], bf16)
                for k in range(4):
                    nc.sync.dma_start(wf_sb[:, k, :], wfuse[k * 128:(k + 1) * 128, :])
                combT = [fp.tile([128, npad], bf16, tag=f"ct{k}", name=f"ct{k}")
                         for k in range(4)]
                for k in range(4):
                    nc.sync.dma_start_transpose(combT[k][:], combined[:, k * 128:(k + 1) * 128])
                acc = fp.tile([128, 512], f32)
                acc2 = fp.tile([128, 512], f32)
                nc.vector.memset(acc[:], 0.0)
                nc.vector.memset(acc2[:], 0.0)
                fused = fp.tile([128, nb, 512], bf16)
                with tc.tile_pool(name="fw", bufs=3) as fw:
                    for b in range(nb):
                        pf = pp.tile([128, 512], f32, tag="pf", bufs=2)
                        for k in range(4):
                            nc.tensor.matmul(pf[:], combT[k][:, b * 128:(b + 1) * 128],
                                             wf_sb[:, k, :], start=(k == 0), stop=(k == 3))
                        nc.scalar.activation(fused[:, b, :], pf[:], Act.Copy)
                        pfs = fw.tile([128, 512], f32, tag="pfs")
                        nc.vector.tensor_copy(pfs[:], pf[:])
                        sq = fw.tile([128, 512], f32, tag="sq")
                        nc.vector.tensor_tensor(sq[:], pfs[:], pfs[:], Alu.mult)
                        nc.vector.tensor_tensor(acc[:], acc[:], pfs[:], Alu.add)
                        nc.vector.tensor_tensor(acc2[:], acc2[:], sq[:], Alu.add)
                    # partition-reduce stats via ones matmul
                    ones = fw.tile([128, 1], f32, tag="ones")
                    nc.vector.memset(ones[:], 1.0)
                    stat = fw.tile([128, 64], f32, tag="stat")
                    nc.vector.memset(stat[:], 0.0)
                    for k in range(4):
                        psk = pp.tile([128, 1], f32, tag="psk")
                        nc.tensor.matmul(psk[:], acc[:, k * 128:(k + 1) * 128], ones[:])
                        nc.vector.tensor_copy(stat[:, k:k + 1], psk[:])
                        psk2 = pp.tile([128, 1], f32, tag="psk")
                        nc.tensor.matmul(psk2[:], acc2[:, k * 128:(k + 1) * 128], ones[:])
                        nc.vector.tensor_copy(stat[:, 32 + k:33 + k], psk2[:])
                    nc.sync.dma_start(ccin[:], stat[:])
                    sg_sb = fw.tile([128, 64], f32, tag="sg")
                    if USE_CC:
                        nc.gpsimd.collective_compute(
                            "AllReduce", Alu.add, replica_groups=[list(range(NCORES))],
                            ins=[ccin[:]], outs=[ccout[:]])
                        nc.sync.dma_start(sg_sb[:], ccout[:])
                    else:
                        nc.sync.dma_start(sg_sb[:], ccin[:])
                    # transpose stats to row layout
                    id_sb = fw.tile([128, 128], f32, tag="id")
                    nc.sync.dma_start(id_sb[:], ident[:])
                    pt1 = pp.tile([32, 128], f32, tag="pt1")
                    nc.tensor.transpose(pt1[:], sg_sb[:, 0:32], id_sb[:])
                    pt2 = pp.tile([32, 128], f32, tag="pt2")
                    nc.tensor.transpose(pt2[:], sg_sb[:, 32:64], id_sb[:])
                    gam_t = fw.tile([32, 128], f32, tag="gam")
                    nc.sync.dma_start(gam_t[:], bnpg[:])
                    bet_t = fw.tile([32, 128], f32, tag="bet")
                    nc.sync.dma_start(bet_t[:], bnpb[:])
                    m = fw.tile([32, 128], f32, tag="m")
                    nc.vector.tensor_scalar_mul(m[:], pt1[:], 1.0 / n_nodes)
                    e2 = fw.tile([32, 128], f32, tag="e2")
                    nc.vector.tensor_scalar_mul(e2[:], pt2[:], 1.0 / n_nodes)
                    msq = fw.tile([32, 128], f32, tag="msq")
                    nc.vector.tensor_tensor(msq[:], m[:], m[:], Alu.mult)
                    var = fw.tile([32, 128], f32, tag="var")
                    nc.vector.tensor_tensor(var[:], e2[:], msq[:], Alu.subtract)
                    nc.vector.tensor_scalar_add(var[:], var[:], BN_EPS)
                    sd = fw.tile([32, 128], f32, tag="sd")
                    nc.scalar.activation(sd[:], var[:], Act.Sqrt)
                    rs = fw.tile([32, 128], f32, tag="rs")
                    nc.vector.reciprocal(rs[:], sd[:])
                    A = fw.tile([32, 128], f32, tag="A")
                    nc.vector.tensor_tensor(A[:], rs[:], gam_t[:], Alu.mult)
                    mA = fw.tile([32, 128], f32, tag="mA")
                    nc.vector.tensor_tensor(mA[:], m[:], A[:], Alu.mult)
                    B = fw.tile([32, 128], f32, tag="B")
                    nc.vector.tensor_tensor(B[:], bet_t[:], mA[:], Alu.subtract)
                    nc.sync.dma_start(abtmp[0:4, :], A[0:4, :])
                    nc.sync.dma_start(abtmp[4:8, :], B[0:4, :])
                    ab_sb = fw.tile([1, 1024], f32, tag="ab")
                    nc.sync.dma_start(ab_sb[:], abtmp.rearrange("a b -> (a b)")[None, :])
                    ones1 = fw.tile([1, 128], f32, tag="o1")
                    nc.vector.memset(ones1[:], 1.0)
                    pA = pp.tile([128, 512], f32, tag="pA")
                    nc.tensor.matmul(pA[:], ones1[:], ab_sb[:, 0:512])
                    pB = pp.tile([128, 512], f32, tag="pB")
                    nc.tensor.matmul(pB[:], ones1[:], ab_sb[:, 512:1024])
                    for b in range(nb):
                        t0 = fw.tile([128, 512], f32, tag="t0")
                        nc.scalar.activation(t0[:], fused[:, b, :], Act.Copy)
                        t1 = fw.tile([128, 512], f32, tag="t1")
                        nc.vector.tensor_tensor(t1[:], t0[:], pA[:], Alu.mult)
                        nc.vector.tensor_tensor(t1[:], t1[:], pB[:], Alu.add)
                        nc.vector.tensor_scalar_max(t1[:], t1[:], 0.0)
                        rows = min(128, npc - b * 128)
                        nc.sync.dma_start(out_d[b * 128:b * 128 + rows, :], t1[0:rows, :])
    nc.compile()
    return nc


def kernel(**inputs):
    x = np.asarray(inputs["x"], dtype=np.float32)
    ei = np.asarray(inputs["edge_index"])
    n_nodes, D = x.shape
    npc, nb, half, trows_lo, trows_hi = _derive(n_nodes)
    n_all = ((n_nodes + 127) // 128) * 128
    npad = nb * 128

    def g(name):
        return np.asarray(inputs[name], dtype=np.float32)

    W_f, W_b = g("W_f"), g("W_b")
    asf, adf = g("att_src_f"), g("att_dst_f")
    asb, adb = g("att_src_b"), g("att_dst_b")
    W_fuse = g("W_fuse")
    gamma, beta = g("bn_gamma"), g("bn_beta")

    wall = np.zeros((512, 528), dtype=np.float32)
    wall[:, 0:256] = W_f.reshape(512, 256)
    wall[:, 256:260] = np.einsum("dhc,hc->dh", W_f, asf)
    wall[:, 260:516] = W_b.reshape(512, 256)
    wall[:, 516:520] = np.einsum("dhc,hc->dh", W_b, asb)
    wall[:, 520:524] = np.einsum("dhc,hc->dh", W_f, adf)
    wall[:, 524:528] = np.einsum("dhc,hc->dh", W_b, adb)

    xT = np.zeros((512, n_all), dtype=ml_dtypes.bfloat16)
    xT[:, :n_nodes] = x.T
    drow = np.zeros((1, 384), dtype=ml_dtypes.bfloat16)
    drow[0, 256:260] = DUMMY_AS
    iota = np.broadcast_to(np.arange(128), (128, 128)).astype(ml_dtypes.bfloat16)
    ident = np.eye(128, dtype=np.float32)
    bnpg = np.zeros((32, 128), dtype=np.float32); bnpg[0:4] = gamma.reshape(4, 128)
    bnpb = np.zeros((32, 128), dtype=np.float32); bnpb[0:4] = beta.reshape(4, 128)

    src, dst = ei[0].astype(np.int64), ei[1].astype(np.int64)
    clo_f, chi_f, g1f, g2f, dpf = _prep_edges(src, dst, n_nodes)
    clo_b, chi_b, g1b, g2b, dpb = _prep_edges(dst, src, n_nodes)

    nc = _build_program(n_nodes, clo_f, chi_f, clo_b, chi_b)

    in_maps = []
    for c in range(NCORES):
        xTo = np.zeros((512, npad), dtype=ml_dtypes.bfloat16)
        xTo[:, :npc] = x.T[:, c * npc:(c + 1) * npc]
        in_maps.append({
            "xT": xT, "xTo": xTo,
            "wall": wall.astype(ml_dtypes.bfloat16),
            "wfuse": W_fuse.astype(ml_dtypes.bfloat16),
            "drow": drow, "iota": iota, "ident": ident, "bnpg": bnpg, "bnpb": bnpb,
            "g1f": g1f[c], "g2f": g2f[c], "dpf": dpf[c].astype(np.float32),
            "g1b": g1b[c], "g2b": g2b[c], "dpb": dpb[c].astype(np.float32),
        })
    kernel.last_nc = nc
    res = run_bass_kernel_spmd(nc, in_maps, list(range(NCORES)))
    out = np.concatenate([np.asarray(res.results[c]["out"]) for c in range(NCORES)], axis=0)
    return out[:n_nodes].astype(np.float32)


if __name__ == "__main__":
    pass



# revision 17
# speedup vs baseline: 1.7332x; 1.7332x over previous
"""Bidirectional GATConv + fusion + BatchNorm + ReLU on 8 Trainium2 cores.

Strategy: nodes sharded 8 ways by aggregation target. Each core:
  1. projects x -> per-direction gather tables with interleaved layout
     [h0|1|h1|1|h2|1|h3|1|a_s] (768B rows), plus a local a_d table.
  2. walks its incident edges (dst-sorted, host-partitioned) in 128-edge
     chunks: dma_gather of source rows + a_d rows, batched exp(lrelu) of
     logits, then per chunk four fused one-hot-times-exp builds (DVE
     dual-scalar is_equal*mult) feeding four 65-row matmuls that
     scatter-add messages and softmax denominators into PSUM.
  3. fuses [fwd|bwd] @ W_fuse, computes BN stats via PSUM-accumulated
     ones-matmuls, AllReduces them (4KB), normalizes + ReLU.
Biases provably cancel through BatchNorm and are dropped.
"""
import sys

sys.path.insert(0, "/opt/trn_rl_repo")

import numpy as np
import ml_dtypes

import concourse.bass as bass
import concourse.bacc as bacc
import concourse.mybir as mybir
from concourse import tile
from concourse import library_config
from concourse.bass_utils import run_bass_kernel_spmd

bf16 = mybir.dt.bfloat16
f32 = mybir.dt.float32
i16 = mybir.dt.int16
Alu = mybir.AluOpType
Act = mybir.ActivationFunctionType

NCORES = 8
USE_CC = __import__("os").environ.get("NO_CC", "0") != "1"
KB = 3          # dst blocks per gather supergroup
NEG_SLOPE = 0.2
BN_EPS = 1e-5
DUMMY_AS = -60.0


def _derive(n_nodes):
    npc = n_nodes // NCORES
    nb = (npc + 127) // 128
    half = ((n_nodes // 2) // 128) * 128
    trows_lo = half + 64           # dummy row at index half+16
    trows_hi = (n_nodes - half) + NCORES * 16 + 64
    return npc, nb, half, trows_lo, trows_hi


def _pack_idx(arr):
    """int16 [n] (n%16==0) -> [128, n/16] wrapped in 16 partitions, replicated per Q7 core."""
    a = arr.reshape(-1, 16).T
    return np.tile(a, (8, 1)).astype(np.int16)


def _prep_edges(gidx, anode, n_nodes):
    """Host edge partitioning for one direction.

    gidx: gather-side node per edge; anode: aggregation node per edge.
    Returns per-core chunk tensors with a uniform (CLO, CHI) template.
    """
    npc, nb, half, _, _ = _derive(n_nodes)
    n_all = ((n_nodes + 127) // 128) * 128
    dlo, dhi = half + 16, (n_all - half) + 16
    core = anode // npc
    local = anode - core * npc
    block = local // 128
    dstpos = local % 128
    hi = (gidx >= half).astype(np.int64)
    lidx = gidx - hi * half

    counts = np.zeros((NCORES, nb, 2), dtype=np.int64)
    np.add.at(counts, (core, block, hi), 1)
    nchunks = -(-counts // 128)  # ceil
    clo = int(nchunks[:, :, 0].max())
    chi = int(nchunks[:, :, 1].max())

    order = np.lexsort((lidx, hi, block, core))
    g_s, blk_s, hi_s, lidx_s, dp_s = (
        core[order], block[order], hi[order], lidx[order], dstpos[order])

    cmax = max(clo, chi)
    g1 = np.empty((NCORES, nb, 2, cmax * 128), dtype=np.int16)
    g1[:, :, 0, :] = dlo
    g1[:, :, 1, :] = dhi
    g2 = np.zeros((NCORES, nb, 2, cmax * 128), dtype=np.int16)
    dp = np.full((NCORES, nb, 2, cmax * 128), 200.0, dtype=np.float32)

    # fill positions within each (core, block, hi) group
    flat_grp = (g_s * nb + blk_s) * 2 + hi_s
    idx_sorted = np.argsort(flat_grp, kind="stable")
    fg = flat_grp[idx_sorted]
    pos = np.arange(len(fg)) - np.concatenate(([0], np.cumsum(np.bincount(fg, minlength=NCORES*nb*2))))[fg]
    c_, b_, h_ = fg // (nb * 2), (fg // 2) % nb, fg % 2
    g1[c_, b_, h_, pos] = lidx_s[idx_sorted].astype(np.int16)
    g2[c_, b_, h_, pos] = (blk_s[idx_sorted] * 128 + dp_s[idx_sorted]).astype(np.int16)
    g2[:, :, :, :][g2 < 0] = 0
    # pad g2 entries point at the block's first row
    padmask = np.ones((NCORES, nb, 2, cmax * 128), dtype=bool)
    padmask[c_, b_, h_, pos] = False
    bb = np.broadcast_to(np.arange(nb)[None, :, None, None] * 128,
                         (NCORES, nb, 2, cmax * 128))
    g2[padmask] = bb[padmask].astype(np.int16)
    dp[c_, b_, h_, pos] = dp_s[idx_sorted].astype(np.float32)

    # trim halves to their own chunk counts
    g1lo, g1hi = g1[:, :, 0, :clo * 128], g1[:, :, 1, :chi * 128]
    g2lo, g2hi = g2[:, :, 0, :clo * 128], g2[:, :, 1, :chi * 128]
    dplo, dphi = dp[:, :, 0, :clo * 128], dp[:, :, 1, :chi * 128]

    # gather idx streams: per supergroup: lo idxs then hi idxs (block-major)
    g1_streams, g2_streams = [], []
    for c in range(NCORES):
        p1, p2 = [], []
        for bs in range(0, nb, KB):
            be = min(bs + KB, nb)
            p1 += [_pack_idx(g1lo[c, bs:be].ravel()), _pack_idx(g1hi[c, bs:be].ravel())]
            p2 += [_pack_idx(g2lo[c, bs:be].ravel()), _pack_idx(g2hi[c, bs:be].ravel())]
        g1_streams.append(np.concatenate(p1, axis=1))
        g2_streams.append(np.concatenate(p2, axis=1))
    # dstpos stream [128, nb*(clo+chi)] block-major, lo chunks then hi chunks
    dpl = dplo.reshape(NCORES, nb, clo, 128).transpose(0, 3, 1, 2).reshape(NCORES, 128, nb * clo)
    dph = dphi.reshape(NCORES, nb, chi, 128).transpose(0, 3, 1, 2).reshape(NCORES, 128, nb * chi)
    dp_stream = np.concatenate([dpl, dph], axis=2)
    return clo, chi, np.stack(g1_streams), np.stack(g2_streams), np.ascontiguousarray(dp_stream)


def _build_program(n_nodes, clo_f, chi_f, clo_b, chi_b):
    npc, nb, half, trows_lo, trows_hi = _derive(n_nodes)
    npad = nb * 128
    n_all = ((n_nodes + 127) // 128) * 128
    npb = n_all // 128                     # projection node blocks
    dummy_lo = half + 16
    dummy_hi = (n_all - half) + 16
    hblocks = half // 128

    nc = bacc.Bacc(None, target_bir_lowering=False)
    inp = lambda name, shape, dt: nc.declare_dram_parameter(name, shape, dt, isOutput=False)
    xTt = inp("xTt", [npb, 128, 512], bf16)     # pre-tiled x
    xTo = inp("xTo", [nb, 128, 512], bf16)      # pre-tiled local x
    wall = inp("wall", [512, 528], bf16)
    wfuse = inp("wfuse", [512, 512], bf16)
    drow = inp("drow", [1, 384], bf16)
    iota_in = inp("iota", [128, 128], bf16)
    ident = inp("ident", [128, 128], f32)
    bnp = inp("bnp", [8, 128], f32)
    streams = {}
    for d, (clo, chi) in (("f", (clo_f, chi_f)), ("b", (clo_b, chi_b))):
        tc_d = nb * (clo + chi)
        streams["g1" + d] = inp("g1" + d, [128, tc_d * 8], i16)
        streams["g2" + d] = inp("g2" + d, [128, tc_d * 8], i16)
        streams["dp" + d] = inp("dp" + d, [128, tc_d], f32)
    out_d = nc.declare_dram_parameter("out", [npc, 512], f32, isOutput=True)

    tabs = {d: [nc.dram_tensor(f"tab{d}{h}", [tr, 384], bf16)
                for h, tr in (("lo", trows_lo), ("hi", trows_hi))] for d in "fb"}
    adtab = nc.dram_tensor("adtab", [npad, 128], bf16)
    combined = nc.dram_tensor("combined", [npad, 512], bf16)
    ccin = nc.dram_tensor("ccin", [128, 8], f32)
    ccout = nc.dram_tensor("ccout", [128, 8], f32, addr_space="Shared")
    abtmp = nc.dram_tensor("abtmp", [8, 128], f32)

    with tile.TileContext(nc) as tc:
        with tc.tile_pool(name="const", bufs=1) as cpool:
            nc.gpsimd.load_library(library_config.mlp)
            wall_sb = cpool.tile([128, 4, 528], bf16)
            nc.sync.dma_start(
                wall_sb[:], wall[:].rearrange("(k p) c -> p k c", p=128))
            iota_sb = cpool.tile([128, 128], bf16)
            nc.sync.dma_start(iota_sb[:], iota_in[:])
            drow_sb = cpool.tile([1, 384], bf16)
            nc.sync.dma_start(drow_sb[:], drow[:])
            for d in "fb":
                nc.sync.dma_start(tabs[d][0][dummy_lo:dummy_lo + 1, :], drow_sb[:])
                nc.sync.dma_start(tabs[d][1][dummy_hi:dummy_hi + 1, :], drow_sb[:])

            # ---------------- projection ----------------
            G = 4
            with (tc.tile_pool(name="proj", bufs=3) as pj,
                  tc.tile_pool(name="pspj", bufs=2, space="PSUM") as pp):
                for grp in range(0, npb, G):
                    gn = min(G, npb - grp)
                    xt = pj.tile([128, G, 4, 128], bf16, tag="xt")
                    nc.sync.dma_start(
                        xt[:, 0:gn, :, :].rearrange("p g k n -> p g (k n)"),
                        xTt[grp:grp + gn].rearrange("g p c -> p g c"))
                    sts = {d: pj.tile([128, G, 264], bf16, tag="st" + d,
                                      name=f"st{d}_{grp}") for d in "fb"}
                    for j in range(gn):
                        ps = {d: pp.tile([128, 260], f32, tag="pj" + d,
                                         name=f"ps{d}_{grp}_{j}") for d in "fb"}
                        for k in range(4):
                            nc.tensor.matmul(ps["f"][:], xt[:, j, k, :],
                                             wall_sb[:, k, 0:260],
                                             start=(k == 0), stop=(k == 3))
                        for k in range(4):
                            nc.tensor.matmul(ps["b"][:], xt[:, j, k, :],
                                             wall_sb[:, k, 260:520],
                                             start=(k == 0), stop=(k == 3))
                        for d, eng in (("f", nc.scalar), ("b", nc.vector)):
                            st = sts[d]
                            # ones columns at 64,129,194,259
                            nc.vector.memset(
                                st[:, j, 0:260].rearrange("p (g c) -> p g c", c=65)[:, :, 64:65], 1.0)
                            iv = st[:, j, 0:260].rearrange("p (g c) -> p g c", c=65)[:, :, 0:64]
                            src = ps[d][:, 0:256].rearrange("p (g c) -> p g c", c=64)
                            if d == "f":
                                eng.activation(iv, src, Act.Copy)
                                eng.activation(st[:, j, 260:264], ps[d][:, 256:260], Act.Copy)
                            else:
                                eng.tensor_copy(iv, src)
                                eng.tensor_copy(st[:, j, 260:264], ps[d][:, 256:260])
                    # batched table writes (segment by lo/hi boundary)
                    for d in "fb":
                        j0 = 0
                        while j0 < gn:
                            blk0 = grp + j0
                            if blk0 < hblocks:
                                jn = min(gn - j0, hblocks - blk0)
                                dst = tabs[d][0][blk0 * 128:(blk0 + jn) * 128, 0:264]
                            else:
                                jn = gn - j0
                                r0 = (blk0 - hblocks) * 128
                                dst = tabs[d][1][r0:r0 + jn * 128, 0:264]
                            nc.sync.dma_start(
                                dst.rearrange("(g p) c -> p g c", p=128),
                                sts[d][:, j0:j0 + jn, :])
                            j0 += jn
                # local a_d table
                ad_stage = pj.tile([128, nb, 8], bf16, tag="ad")
                for grp in range(0, nb, G):
                    gn = min(G, nb - grp)
                    xo = pj.tile([128, G, 4, 128], bf16, tag="xo")
                    nc.sync.dma_start(
                        xo[:, 0:gn, :, :].rearrange("p g k n -> p g (k n)"),
                        xTo[grp:grp + gn].rearrange("g p c -> p g c"))
                    for j in range(gn):
                        pa = pp.tile([128, 8], f32, tag="pa")
                        for k in range(4):
                            nc.tensor.matmul(pa[:], xo[:, j, k, :], wall_sb[:, k, 520:528],
                                             start=(k == 0), stop=(k == 3))
                        nc.vector.tensor_copy(ad_stage[:, grp + j, :], pa[:])
                nc.sync.dma_start(
                    adtab.rearrange("(b p) c -> p b c", p=128)[:, :, 0:8], ad_stage[:])

            # ---------------- edge passes ----------------
            for d, clo, chi, dcol in (("f", clo_f, chi_f, 0), ("b", clo_b, chi_b, 256)):
                tc_d = nb * (clo + chi)
                with tc.tile_pool(name="edge" + d, bufs=1) as ep:
                    dp_sb = ep.tile([128, tc_d], f32)
                    nc.sync.dma_start(dp_sb[:], streams["dp" + d][:])
                    g1_sb = ep.tile([128, tc_d * 8], i16)
                    nc.sync.dma_start(g1_sb[:], streams["g1" + d][:])
                    g2_sb = ep.tile([128, tc_d * 8], i16)
                    nc.sync.dma_start(g2_sb[:], streams["g2" + d][:])
                    with (tc.tile_pool(name="ew" + d, bufs=2) as ew,
                          tc.tile_pool(name="sew" + d, bufs=3) as sew,
                          tc.tile_pool(name="psed" + d, bufs=4, space="PSUM") as pp):
                        c1 = c2 = 0  # stream column cursors
                        for bs in range(0, nb, KB):
                            kbs = min(KB, nb - bs)
                            nslo, nshi = kbs * clo, kbs * chi
                            # gathers: h rows per half, a_d rows both halves
                            gts = {}
                            for hname, cc, ns, tabx in (("lo", clo, nslo, tabs[d][0]),
                                                        ("hi", chi, nshi, tabs[d][1])):
                                ni = ns * 128
                                gt = ew.tile([128, ns, 384], bf16, tag="g1t" + hname)
                                nc.gpsimd.dma_gather(
                                    gt[:], tabx[:], g1_sb[:, c1:c1 + ni // 16],
                                    num_idxs=ni, num_idxs_reg=ni, elem_size=384,
                                    single_packet=False)
                                c1 += ni // 16
                                gts[hname] = gt
                            nia = (nslo + nshi) * 128
                            at = ew.tile([128, nslo + nshi, 128], bf16, tag="g2t")
                            nc.gpsimd.dma_gather(
                                at[:], adtab[:], g2_sb[:, c2:c2 + nia // 16],
                                num_idxs=nia, num_idxs_reg=nia, elem_size=128,
                                single_packet=False)
                            c2 += nia // 16
                            adofs = dcol // 256 * 4  # f: cols 0:4, b: 4:8
                            # batched logits -> exp
                            exts = {}
                            for hname, ns, aofs in (("lo", nslo, 0), ("hi", nshi, nslo)):
                                if ns == 0:
                                    continue
                                gt = gts[hname]
                                et = ew.tile([128, ns, 4], bf16, tag="et" + hname)
                                nc.vector.tensor_tensor(
                                    et[:], gt[:, :, 260:264],
                                    at[:, aofs:aofs + ns, adofs:adofs + 4], Alu.add)
                                lt = ew.tile([128, ns, 4], bf16, tag="lt" + hname)
                                nc.vector.scalar_tensor_tensor(
                                    lt[:], et[:], NEG_SLOPE, et[:], op0=Alu.mult, op1=Alu.max)
                                ext = ew.tile([128, ns, 4], f32, tag="ext" + hname)
                                nc.scalar.activation(ext[:], lt[:], Act.Exp)
                                exts[hname] = ext
                            # per-block chunk loop
                            for j in range(kbs):
                                b = bs + j
                                pb = pp.tile([128, 260], f32, tag="pb")
                                seq = [("lo", k) for k in range(clo)] + \
                                      [("hi", k) for k in range(chi)]
                                for ci, (hname, k) in enumerate(seq):
                                    cc = clo if hname == "lo" else chi
                                    slot = j * cc + k
                                    dpcol = (b * clo + k) if hname == "lo" \
                                        else (nb * clo + b * chi + k)
                                    gt, ext = gts[hname], exts[hname]
                                    se = sew.tile([128, 4, 128], bf16, tag="se")
                                    for h in range(4):
                                        nc.vector.tensor_scalar(
                                            se[:, h, :], iota_sb[:],
                                            dp_sb[:, dpcol:dpcol + 1],
                                            ext[:, slot, h:h + 1],
                                            op0=Alu.is_equal, op1=Alu.mult)
                                    first, last = ci == 0, ci == len(seq) - 1
                                    for h in range(4):
                                        # start=True zeroes the whole 2KB PSUM
                                        # bank: only h==0 may start it
                                        nc.tensor.matmul(
                                            pb[:, 65 * h:65 * h + 65], se[:, h, :],
                                            gt[:, slot, 65 * h:65 * h + 65],
                                            start=first and h == 0, stop=last,
                                            skip_group_check=True)
                                dn = sew.tile([128, 4], f32, tag="dn")
                                nc.vector.tensor_scalar_add(
                                    dn[:],
                                    pb[:].rearrange("p (g c) -> p g c", c=65)[:, :, 64:65].squeeze(2),
                                    1e-16)
                                rc = sew.tile([128, 4], f32, tag="rc")
                                nc.vector.reciprocal(rc[:], dn[:])
                                ob = sew.tile([128, 256], bf16, tag="ob")
                                for h in range(4):
                                    nc.vector.tensor_scalar(
                                        ob[:, 64 * h:64 * h + 64],
                                        pb[:, 65 * h:65 * h + 64],
                                        rc[:, h:h + 1], None, op0=Alu.mult)
                                nc.sync.dma_start(
                                    combined[b * 128:(b + 1) * 128, dcol:dcol + 256], ob[:])

            # ---------------- fusion + BN ----------------
            with (tc.tile_pool(name="fuse", bufs=1) as fp,
                  tc.tile_pool(name="psfu", bufs=2, space="PSUM") as pp,
                  tc.tile_pool(name="psf1", bufs=1, space="PSUM") as pp1,
                  tc.tile_pool(name="psst", bufs=1, space="PSUM") as pst):
                wf_sb = fp.tile([128, 4, 512], bf16)
                nc.sync.dma_start(
                    wf_sb[:], wfuse[:].rearrange("(k p) c -> p k c", p=128))
                combT = [fp.tile([128, npad], bf16, tag=f"ct{k}", name=f"ct{k}")
                         for k in range(4)]
                for k in range(4):
                    nc.sync.dma_start_transpose(combT[k][:], combined[:, k * 128:(k + 1) * 128])
                onesb = fp.tile([128, 1], bf16)
                nc.vector.memset(onesb[:], 1.0)
                fused = fp.tile([128, nb, 512], bf16)
                statps = pst.tile([128, 8], f32)
                with tc.tile_pool(name="fw", bufs=3) as fw:
                    for b in range(nb):
                        pf = pp.tile([128, 512], f32, tag="pf")
                        for k in range(4):
                            nc.tensor.matmul(pf[:], combT[k][:, b * 128:(b + 1) * 128],
                                             wf_sb[:, k, :], start=(k == 0), stop=(k == 3))
                        nc.scalar.activation(fused[:, b, :], pf[:], Act.Copy)
                        sq = fw.tile([128, 512], bf16, tag="sq")
                        nc.vector.tensor_tensor(sq[:], fused[:, b, :], fused[:, b, :], Alu.mult)
                        for g in range(4):
                            # start only on the very first range-matmul: start
                            # zeroes the whole 2KB PSUM bank
                            nc.tensor.matmul(statps[:, g:g + 1],
                                             fused[:, b, 128 * g:128 * (g + 1)], onesb[:],
                                             start=(b == 0 and g == 0), stop=(b == nb - 1),
                                             skip_group_check=True)
                            nc.tensor.matmul(statps[:, 4 + g:5 + g],
                                             sq[:, 128 * g:128 * (g + 1)], onesb[:],
                                             start=False, stop=(b == nb - 1),
                                             skip_group_check=True)
                    stat_sb = fw.tile([128, 8], f32, tag="stat")
                    nc.vector.tensor_copy(stat_sb[:], statps[:])
                    nc.sync.dma_start(ccin[:], stat_sb[:])
                    sg_sb = fw.tile([128, 8], f32, tag="sg")
                    if USE_CC:
                        nc.gpsimd.collective_compute(
                            "AllReduce", Alu.add, replica_groups=[list(range(NCORES))],
                            ins=[ccin[:]], outs=[ccout[:]])
                        nc.sync.dma_start(sg_sb[:], ccout[:])
                    else:
                        nc.sync.dma_start(sg_sb[:], ccin[:])
                    # transpose stats to row layout: [4, 128] sums / sq-sums
                    id_sb = fw.tile([128, 128], f32, tag="id")
                    nc.sync.dma_start(id_sb[:], ident[:])
                    pt1 = pp1.tile([4, 128], f32, tag="pt1")
                    nc.tensor.transpose(pt1[:], sg_sb[:, 0:4], id_sb[:])
                    pt2 = pp1.tile([4, 128], f32, tag="pt2")
                    nc.tensor.transpose(pt2[:], sg_sb[:, 4:8], id_sb[:])
                    gam = fw.tile([4, 128], f32, tag="gam")
                    nc.sync.dma_start(gam[:], bnp[0:4, :])
                    bet = fw.tile([4, 128], f32, tag="bet")
                    nc.sync.dma_start(bet[:], bnp[4:8, :])
                    m = fw.tile([4, 128], f32, tag="m")
                    nc.vector.tensor_scalar_mul(m[:], pt1[:], 1.0 / n_nodes)
                    e2 = fw.tile([4, 128], f32, tag="e2")
                    nc.vector.tensor_scalar_mul(e2[:], pt2[:], 1.0 / n_nodes)
                    var = fw.tile([4, 128], f32, tag="var")
                    msq = fw.tile([4, 128], f32, tag="msq")
                    nc.vector.tensor_tensor(msq[:], m[:], m[:], Alu.mult)
                    nc.vector.tensor_tensor(var[:], e2[:], msq[:], Alu.subtract)
                    nc.vector.tensor_scalar_add(var[:], var[:], BN_EPS)
                    sd = fw.tile([4, 128], f32, tag="sd")
                    nc.scalar.activation(sd[:], var[:], Act.Sqrt)
                    rs = fw.tile([4, 128], f32, tag="rs")
                    nc.vector.reciprocal(rs[:], sd[:])
                    A = fw.tile([4, 128], f32, tag="A")
                    nc.vector.tensor_tensor(A[:], rs[:], gam[:], Alu.mult)
                    mA = fw.tile([4, 128], f32, tag="mA")
                    nc.vector.tensor_tensor(mA[:], m[:], A[:], Alu.mult)
                    B = fw.tile([4, 128], f32, tag="B")
                    nc.vector.tensor_tensor(B[:], bet[:], mA[:], Alu.subtract)
                    nc.sync.dma_start(abtmp[0:4, :], A[:])
                    nc.sync.dma_start(abtmp[4:8, :], B[:])
                    ab_sb = fw.tile([1, 1024], f32, tag="ab")
                    nc.sync.dma_start(ab_sb[:], abtmp.rearrange("a b -> (a b)")[None, :])
                    ones1 = fw.tile([1, 128], f32, tag="o1")
                    nc.vector.memset(ones1[:], 1.0)
                    pA = pp1.tile([128, 512], f32, tag="pA")
                    nc.tensor.matmul(pA[:], ones1[:], ab_sb[:, 0:512])
                    pB = pp1.tile([128, 512], f32, tag="pB")
                    nc.tensor.matmul(pB[:], ones1[:], ab_sb[:, 512:1024])
                    pAs = fw.tile([128, 512], bf16, tag="pAs")
                    nc.vector.tensor_copy(pAs[:], pA[:])
                    pBs = fw.tile([128, 512], bf16, tag="pBs")
                    nc.vector.tensor_copy(pBs[:], pB[:])
                    for b in range(nb):
                        t1 = fw.tile([128, 512], bf16, tag="t1")
                        nc.vector.tensor_tensor(t1[:], fused[:, b, :], pAs[:], Alu.mult)
                        nc.vector.tensor_tensor(t1[:], t1[:], pBs[:], Alu.add)
                        t2 = fw.tile([128, 512], f32, tag="t2")
                        nc.scalar.activation(t2[:], t1[:], Act.Relu)
                        rows = min(128, npc - b * 128)
                        nc.sync.dma_start(out_d[b * 128:b * 128 + rows, :], t2[0:rows, :])
    nc.compile()
    return nc


def kernel(**inputs):
    x = np.asarray(inputs["x"], dtype=np.float32)
    ei = np.asarray(inputs["edge_index"])
    n_nodes, D = x.shape
    npc, nb, half, trows_lo, trows_hi = _derive(n_nodes)
    n_all = ((n_nodes + 127) // 128) * 128
    npb = n_all // 128
    npad = nb * 128

    def g(name):
        return np.asarray(inputs[name], dtype=np.float32)

    W_f, W_b = g("W_f"), g("W_b")
    asf, adf = g("att_src_f"), g("att_dst_f")
    asb, adb = g("att_src_b"), g("att_dst_b")
    W_fuse = g("W_fuse")
    gamma, beta = g("bn_gamma"), g("bn_beta")

    wall = np.zeros((512, 528), dtype=np.float32)
    wall[:, 0:256] = W_f.reshape(512, 256)
    wall[:, 256:260] = np.einsum("dhc,hc->dh", W_f, asf)
    wall[:, 260:516] = W_b.reshape(512, 256)
    wall[:, 516:520] = np.einsum("dhc,hc->dh", W_b, asb)
    wall[:, 520:524] = np.einsum("dhc,hc->dh", W_f, adf)
    wall[:, 524:528] = np.einsum("dhc,hc->dh", W_b, adb)

    x_pad = np.zeros((n_all, 512), dtype=np.float32)
    x_pad[:n_nodes] = x
    # xTt[blk, p, 128k+n] = x[blk*128+n, 128k+p]
    xTt = np.ascontiguousarray(
        x_pad.reshape(npb, 128, 4, 128).transpose(0, 3, 2, 1)
        .reshape(npb, 128, 512)).astype(ml_dtypes.bfloat16)

    drow = np.zeros((1, 384), dtype=ml_dtypes.bfloat16)
    drow[0, 260:264] = DUMMY_AS
    iota = np.broadcast_to(np.arange(128), (128, 128)).astype(ml_dtypes.bfloat16)
    ident = np.eye(128, dtype=np.float32)
    bnp = np.zeros((8, 128), dtype=np.float32)
    bnp[0:4] = gamma.reshape(4, 128)
    bnp[4:8] = beta.reshape(4, 128)

    src, dst = ei[0].astype(np.int64), ei[1].astype(np.int64)
    clo_f, chi_f, g1f, g2f, dpf = _prep_edges(src, dst, n_nodes)
    clo_b, chi_b, g1b, g2b, dpb = _prep_edges(dst, src, n_nodes)

    nc = _build_program(n_nodes, clo_f, chi_f, clo_b, chi_b)

    in_maps = []
    for c in range(NCORES):
        xo_pad = np.zeros((npad, 512), dtype=np.float32)
        xo_pad[:npc] = x[c * npc:(c + 1) * npc]
        xTo = np.ascontiguousarray(
            xo_pad.reshape(nb, 128, 4, 128).transpose(0, 3, 2, 1)
            .reshape(nb, 128, 512)).astype(ml_dtypes.bfloat16)
        in_maps.append({
            "xTt": xTt, "xTo": xTo,
            "wall": wall.astype(ml_dtypes.bfloat16),
            "wfuse": W_fuse.astype(ml_dtypes.bfloat16),
            "drow": drow, "iota": iota, "ident": ident, "bnp": bnp,
            "g1f": g1f[c], "g2f": g2f[c], "dpf": dpf[c],
            "g1b": g1b[c], "g2b": g2b[c], "dpb": dpb[c],
        })
    kernel.last_nc = nc
    res = run_bass_kernel_spmd(nc, in_maps, list(range(NCORES)))
    out = np.concatenate([np.asarray(res.results[c]["out"]) for c in range(NCORES)], axis=0)
    return out[:n_nodes].astype(np.float32)


if __name__ == "__main__":
    pass
